# revision 5
# baseline (speedup 1.0000x reference)
"""Attention-pooling kernel for Trainium2 (Bass/Tile), 8-core data-parallel.

Computes, per sample b:
    energy  = tanh(x[b] @ w + bias)          # [S]
    weights = softmax(energy)                # [S]
    context = weights @ x[b]                 # [H]

Single pass over HBM: because energy = tanh(...) is bounded in [-1, 1],
softmax needs no max-subtraction, so exp/sum/weighted-sum all happen while
each x-tile is resident in SBUF.

Layout per core (B_PER=8 samples, S=2048, H=1024):
  - x streamed in chunks of CHUNK_S=512 rows: SBUF tile [128, J*H] where
    partition p holds rows s = s0 + j*128 + p (j = 0..J-1). 4KiB-contiguous
    HBM runs, and the per-sample u matrix [128, 16] has s = 128*q + p, which
    makes the transposed weights output contiguous in HBM.
  - energy: one fused scalar_tensor_tensor (VectorE, accum_out) per [128, 1024] subtile.
  - tanh/exp: ScalarE (both in the exp_and_others table set).
  - context: TensorE matmuls (float32r) accumulating into PSUM [1, 1024].
  - usum: ones^T @ u matmul -> [16,16] PSUM -> free-dim reduce -> reciprocal.
"""

import sys

if "/opt/trn_rl_repo" not in sys.path:
    sys.path.insert(0, "/opt/trn_rl_repo")

import numpy as np

import concourse.bass as bass  # noqa: F401  (engine types referenced via nc)
import concourse.tile as tile
from concourse import bacc, mybir
from concourse.bass_utils import run_bass_kernel_spmd

B, S, H = 64, 2048, 1024
N_CORES = 8
B_PER = B // N_CORES

F32 = mybir.dt.float32
F32R = mybir.dt.float32r


def build_nc(b_per=B_PER, s=S, h=H, chunk_s=512, x_bufs=4):
    """Build + compile the per-core Bass kernel. Same program on all cores."""
    assert s % 128 == 0 and chunk_s % 128 == 0 and s % chunk_s == 0
    assert h % 512 == 0
    J = chunk_s // 128          # subtiles per chunk
    Q = s // 128                # subtiles per sample
    NCH = s // chunk_s          # chunks per sample
    NB = h // 512               # 512-wide matmul slices per subtile

    nc = bacc.Bacc(None, target_bir_lowering=False)

    x_in = nc.declare_dram_parameter("x", [b_per, s, h], F32R, isOutput=False)
    w_in = nc.declare_dram_parameter("wb", [128, h], F32, isOutput=False)
    b_in = nc.declare_dram_parameter("bb", [128, 1], F32, isOutput=False)
    ones_in = nc.declare_dram_parameter("ones", [128, 16], F32R, isOutput=False)
    id_in = nc.declare_dram_parameter("ident", [128, 128], F32R, isOutput=False)
    ctx_out = nc.declare_dram_parameter("ctx", [b_per, h], F32, isOutput=True)
    wts_out = nc.declare_dram_parameter("wts", [b_per, s], F32, isOutput=True)

    with tile.TileContext(nc) as tc:
        with (
            tc.tile_pool(name="singles", bufs=1) as singles,
            tc.tile_pool(name="xpool", bufs=x_bufs) as xpool,
            tc.tile_pool(name="scratch", bufs=2) as scratch,
            tc.tile_pool(name="small", bufs=4) as small,
            tc.tile_pool(name="upool", bufs=2) as upool,
            tc.tile_pool(name="outp", bufs=2) as outp,
            tc.tile_pool(name="ps_ctx", bufs=2, space="PSUM") as ps_ctx_pool,
            tc.tile_pool(name="ps_us", bufs=2, space="PSUM") as ps_us_pool,
            tc.tile_pool(name="ps_ut", bufs=2, space="PSUM") as ps_ut_pool,
        ):
            w_tile = singles.tile([128, h], F32)
            nc.sync.dma_start(out=w_tile[:, :], in_=w_in[:, :])
            b_tile = singles.tile([128, 1], F32)
            nc.sync.dma_start(out=b_tile[:, :], in_=b_in[:, :])
            ones16 = singles.tile([128, 16], F32R)
            nc.sync.dma_start(out=ones16[:, :], in_=ones_in[:, :])
            ident = singles.tile([128, 128], F32R)
            nc.sync.dma_start(out=ident[:, :], in_=id_in[:, :])

            for b in range(b_per):
                u_s = upool.tile([128, Q], F32R)          # exp(tanh(e)) per sample
                psum_ctx = ps_ctx_pool.tile([1, h], F32)  # unnormalized context

                for c in range(NCH):
                    xt = xpool.tile([128, J, h], F32R)
                    src = x_in[b, c * chunk_s:(c + 1) * chunk_s, :].rearrange(
                        "(j p) h -> p j h", p=128
                    )
                    nc.sync.dma_start(out=xt[:, :, :], in_=src)

                    e_t = small.tile([128, J], F32, tag="e_t")
                    for j in range(J):
                        scr = scratch.tile([128, h], F32)
                        nc.vector.scalar_tensor_tensor(
                            out=scr[:, :],
                            in0=xt[:, j, :].bitcast(F32),
                            scalar=1.0,
                            in1=w_tile[:, :],
                            op0=mybir.AluOpType.mult,
                            op1=mybir.AluOpType.mult,
                            accum_out=e_t[:, j:j + 1],
                        )
                    t_t = small.tile([128, J], F32, tag="t_t")
                    nc.scalar.activation(
                        out=t_t[:, :], in_=e_t[:, :],
                        func=mybir.ActivationFunctionType.Tanh,
                        bias=b_tile[:, :], scale=1.0,
                    )
                    nc.scalar.activation(
                        out=u_s[:, c * J:(c + 1) * J], in_=t_t[:, :],
                        func=mybir.ActivationFunctionType.Exp,
                    )

                    # context accumulation: psum_ctx[0, :] += u_q^T @ x_q
                    for j in range(J):
                        q = c * J + j
                        uq = u_s[:, q:q + 1]
                        for nb in range(NB):
                            nc.tensor.matmul(
                                out=psum_ctx[:, nb * 512:(nb + 1) * 512],
                                lhsT=uq,
                                rhs=xt[:, j, nb * 512:(nb + 1) * 512],
                                start=(q == 0),
                                stop=(q == Q - 1),
                            )

                # --- sample tail ---
                # usum on 16 partitions: ones^T @ u -> [16, Q] (all rows equal)
                psum_us = ps_us_pool.tile([16, Q], F32)
                nc.tensor.matmul(
                    out=psum_us[:, :],
                    lhsT=ones16[:, 0:16],
                    rhs=u_s[:, :],
                    start=True, stop=True,
                )
                usum16 = small.tile([16, 1], F32, tag="usum")
                nc.vector.reduce_sum(
                    out=usum16[:, :], in_=psum_us[:, :], axis=mybir.AxisListType.X
                )
                inv16 = small.tile([16, 1], F32, tag="inv")
                nc.vector.reciprocal(out=inv16[:, :], in_=usum16[:, :])

                # normalized weights: transpose u then scale rows by 1/usum
                psum_ut = ps_ut_pool.tile([Q, 128], F32R)
                nc.tensor.transpose(
                    out=psum_ut[:, :], in_=u_s[:, :], identity=ident[:, :]
                )
                wT = outp.tile([Q, 128], F32, tag="wT")
                nc.vector.tensor_scalar_mul(
                    out=wT[:, :], in0=psum_ut[:, :].bitcast(F32), scalar1=inv16[0:Q, :]
                )
                nc.sync.dma_start(
                    out=wts_out[b].rearrange("(q p) -> q p", p=128), in_=wT[:, :]
                )

                # normalized context
                ctx_sb = outp.tile([1, h], F32, tag="ctx")
                nc.scalar.activation(
                    out=ctx_sb[:, :], in_=psum_ctx[:, :],
                    func=mybir.ActivationFunctionType.Copy,
                    scale=inv16[0:1, :],
                )
                nc.sync.dma_start(out=ctx_out[b:b + 1, :], in_=ctx_sb[:, :])

    nc.compile()
    return nc


_NC_CACHE = {}


def _get_nc():
    if "nc" not in _NC_CACHE:
        _NC_CACHE["nc"] = build_nc()
    return _NC_CACHE["nc"]


ONES_NP = np.ones((128, 16), dtype=np.float32)
IDENT_NP = np.eye(128, dtype=np.float32)


def make_in_maps(rnn_output, attn_w, attn_b):
    x = np.ascontiguousarray(np.asarray(rnn_output, dtype=np.float32))
    w = np.asarray(attn_w, dtype=np.float32).reshape(-1)
    bias = np.float32(np.asarray(attn_b).reshape(()))
    wb = np.ascontiguousarray(np.broadcast_to(w[None, :], (128, H)))
    bb = np.full((128, 1), bias, dtype=np.float32)
    in_maps = []
    for c in range(N_CORES):
        in_maps.append(
            {
                "x": x[c * B_PER:(c + 1) * B_PER], "wb": wb, "bb": bb,
                "ones": ONES_NP, "ident": IDENT_NP,
            }
        )
    return in_maps


def kernel(rnn_output, attn_w, attn_b):
    nc = _get_nc()
    in_maps = make_in_maps(rnn_output, attn_w, attn_b)
    res = run_bass_kernel_spmd(nc, in_maps, list(range(N_CORES)))
    ctx = np.concatenate([res.results[c]["ctx"] for c in range(N_CORES)], axis=0)
    wts = np.concatenate([res.results[c]["wts"] for c in range(N_CORES)], axis=0)
    return ctx, wts


if __name__ == "__main__":
    rng = np.random.default_rng(0)
    x = rng.standard_normal((B, S, H), dtype=np.float32)
    w = rng.standard_normal(H, dtype=np.float32) * (1.0 / np.sqrt(H))
    b = np.float32(0.01)
    ctx, wts = kernel(x, w, b)
    print(ctx.shape, wts.shape)


# revision 6
# speedup vs baseline: 53.7181x; 53.7181x over previous
"""Attention-pooling kernel for Trainium2 (Bass/Tile), 8-core data-parallel.

Computes, per sample b:
    energy  = tanh(x[b] @ w + bias)          # [S]
    weights = softmax(energy)                # [S]
    context = weights @ x[b]                 # [H]

Single pass over HBM: because energy = tanh(...) is bounded in [-1, 1],
softmax needs no max-subtraction, so exp/sum/weighted-sum all happen while
each x-tile is resident in SBUF.

Layout per core (B_PER=8 samples, S=2048, H=1024):
  - x streamed in chunks of CHUNK_S=512 rows: SBUF tile [128, J*H] where
    partition p holds rows s = s0 + j*128 + p (j = 0..J-1). 4KiB-contiguous
    HBM runs, and the per-sample u matrix [128, 16] has s = 128*q + p, which
    makes the transposed weights output contiguous in HBM.
  - energy: one fused scalar_tensor_tensor (VectorE, accum_out) per [128, 1024] subtile.
  - tanh/exp: ScalarE (both in the exp_and_others table set).
  - context: TensorE matmuls (float32r) accumulating into PSUM [1, 1024].
  - usum: ones^T @ u matmul -> [16,16] PSUM -> free-dim reduce -> reciprocal.
"""

import sys

if "/opt/trn_rl_repo" not in sys.path:
    sys.path.insert(0, "/opt/trn_rl_repo")

import numpy as np

import concourse.bass as bass  # noqa: F401  (engine types referenced via nc)
import concourse.tile as tile
from concourse import bacc, mybir
from concourse.bass_utils import run_bass_kernel_spmd

B, S, H = 64, 2048, 1024
N_CORES = 8
B_PER = B // N_CORES

F32 = mybir.dt.float32
F32R = mybir.dt.float32r


def build_nc(b_per=B_PER, s=S, h=H, chunk_s=512, x_bufs=4, reps=1):
    """Build + compile the per-core Bass kernel. Same program on all cores."""
    assert s % 128 == 0 and chunk_s % 128 == 0 and s % chunk_s == 0
    assert h % 512 == 0
    J = chunk_s // 128          # subtiles per chunk
    Q = s // 128                # subtiles per sample
    NCH = s // chunk_s          # chunks per sample
    NB = h // 512               # 512-wide matmul slices per subtile

    nc = bacc.Bacc(None, target_bir_lowering=False)

    x_in = nc.declare_dram_parameter("x", [b_per, s, h], F32R, isOutput=False)
    w_in = nc.declare_dram_parameter("wb", [128, h], F32, isOutput=False)
    b_in = nc.declare_dram_parameter("bb", [128, 1], F32, isOutput=False)
    ones_in = nc.declare_dram_parameter("ones", [128, 16], F32R, isOutput=False)
    id_in = nc.declare_dram_parameter("ident", [128, 128], F32R, isOutput=False)
    ctx_out = nc.declare_dram_parameter("ctx", [b_per, h], F32, isOutput=True)
    wts_out = nc.declare_dram_parameter("wts", [b_per, s], F32, isOutput=True)

    with tile.TileContext(nc) as tc:
        with (
            tc.tile_pool(name="singles", bufs=1) as singles,
            tc.tile_pool(name="xpool", bufs=x_bufs) as xpool,
            tc.tile_pool(name="scratch", bufs=2) as scratch,
            tc.tile_pool(name="small", bufs=4) as small,
            tc.tile_pool(name="upool", bufs=2) as upool,
            tc.tile_pool(name="outp", bufs=2) as outp,
            tc.tile_pool(name="ps_ctx", bufs=2, space="PSUM") as ps_ctx_pool,
            tc.tile_pool(name="ps_us", bufs=2, space="PSUM") as ps_us_pool,
            tc.tile_pool(name="ps_ut", bufs=2, space="PSUM") as ps_ut_pool,
        ):
            w_tile = singles.tile([128, h], F32)
            nc.sync.dma_start(out=w_tile[:, :], in_=w_in[:, :])
            b_tile = singles.tile([128, 1], F32)
            nc.sync.dma_start(out=b_tile[:, :], in_=b_in[:, :])
            ones16 = singles.tile([128, 16], F32R)
            nc.sync.dma_start(out=ones16[:, :], in_=ones_in[:, :])
            ident = singles.tile([128, 128], F32R)
            nc.sync.dma_start(out=ident[:, :], in_=id_in[:, :])

            for b in [bb for _ in range(reps) for bb in range(b_per)]:
                u_s = upool.tile([128, Q], F32R)          # exp(tanh(e)) per sample
                psum_ctx = ps_ctx_pool.tile([1, h], F32)  # unnormalized context

                for c in range(NCH):
                    xt = xpool.tile([128, J, h], F32R)
                    src = x_in[b, c * chunk_s:(c + 1) * chunk_s, :].rearrange(
                        "(j p) h -> p j h", p=128
                    )
                    nc.sync.dma_start(out=xt[:, :, :], in_=src)

                    e_t = small.tile([128, J], F32, tag="e_t")
                    for j in range(J):
                        scr = scratch.tile([128, h], F32)
                        nc.vector.scalar_tensor_tensor(
                            out=scr[:, :],
                            in0=xt[:, j, :].bitcast(F32),
                            scalar=1.0,
                            in1=w_tile[:, :],
                            op0=mybir.AluOpType.mult,
                            op1=mybir.AluOpType.mult,
                            accum_out=e_t[:, j:j + 1],
                        )
                    t_t = small.tile([128, J], F32, tag="t_t")
                    nc.scalar.activation(
                        out=t_t[:, :], in_=e_t[:, :],
                        func=mybir.ActivationFunctionType.Tanh,
                        bias=b_tile[:, :], scale=1.0,
                    )
                    nc.scalar.activation(
                        out=u_s[:, c * J:(c + 1) * J], in_=t_t[:, :],
                        func=mybir.ActivationFunctionType.Exp,
                    )

                    # context accumulation: psum_ctx[0, :] += u_q^T @ x_q
                    for j in range(J):
                        q = c * J + j
                        uq = u_s[:, q:q + 1]
                        for nb in range(NB):
                            nc.tensor.matmul(
                                out=psum_ctx[:, nb * 512:(nb + 1) * 512],
                                lhsT=uq,
                                rhs=xt[:, j, nb * 512:(nb + 1) * 512],
                                start=(q == 0),
                                stop=(q == Q - 1),
                            )

                # --- sample tail ---
                # usum on 16 partitions: ones^T @ u -> [16, Q] (all rows equal)
                psum_us = ps_us_pool.tile([16, Q], F32)
                nc.tensor.matmul(
                    out=psum_us[:, :],
                    lhsT=ones16[:, 0:16],
                    rhs=u_s[:, :],
                    start=True, stop=True,
                )
                usum16 = small.tile([16, 1], F32, tag="usum")
                nc.vector.reduce_sum(
                    out=usum16[:, :], in_=psum_us[:, :], axis=mybir.AxisListType.X
                )
                inv16 = small.tile([16, 1], F32, tag="inv")
                nc.vector.reciprocal(out=inv16[:, :], in_=usum16[:, :])

                # normalized weights: transpose u then scale rows by 1/usum
                psum_ut = ps_ut_pool.tile([Q, 128], F32R)
                nc.tensor.transpose(
                    out=psum_ut[:, :], in_=u_s[:, :], identity=ident[:, :]
                )
                wT = outp.tile([Q, 128], F32, tag="wT")
                nc.vector.tensor_scalar_mul(
                    out=wT[:, :], in0=psum_ut[:, :].bitcast(F32), scalar1=inv16[0:Q, :]
                )
                nc.sync.dma_start(
                    out=wts_out[b].rearrange("(q p) -> q p", p=128), in_=wT[:, :]
                )

                # normalized context
                ctx_sb = outp.tile([1, h], F32, tag="ctx")
                nc.scalar.activation(
                    out=ctx_sb[:, :], in_=psum_ctx[:, :],
                    func=mybir.ActivationFunctionType.Copy,
                    scale=inv16[0:1, :],
                )
                nc.sync.dma_start(out=ctx_out[b:b + 1, :], in_=ctx_sb[:, :])

    nc.compile()
    return nc


_NC_CACHE = {}


def _get_nc():
    if "nc" not in _NC_CACHE:
        _NC_CACHE["nc"] = build_nc()
    return _NC_CACHE["nc"]


ONES_NP = np.ones((128, 16), dtype=np.float32)
IDENT_NP = np.eye(128, dtype=np.float32)


def make_in_maps(rnn_output, attn_w, attn_b):
    x = np.ascontiguousarray(np.asarray(rnn_output, dtype=np.float32))
    w = np.asarray(attn_w, dtype=np.float32).reshape(-1)
    bias = np.float32(np.asarray(attn_b).reshape(()))
    wb = np.ascontiguousarray(np.broadcast_to(w[None, :], (128, H)))
    bb = np.full((128, 1), bias, dtype=np.float32)
    in_maps = []
    for c in range(N_CORES):
        in_maps.append(
            {
                "x": x[c * B_PER:(c + 1) * B_PER], "wb": wb, "bb": bb,
                "ones": ONES_NP, "ident": IDENT_NP,
            }
        )
    return in_maps


def kernel(rnn_output, attn_w, attn_b):
    nc = _get_nc()
    in_maps = make_in_maps(rnn_output, attn_w, attn_b)
    res = run_bass_kernel_spmd(nc, in_maps, list(range(N_CORES)))
    ctx = np.concatenate([res.results[c]["ctx"] for c in range(N_CORES)], axis=0)
    wts = np.concatenate([res.results[c]["wts"] for c in range(N_CORES)], axis=0)
    return ctx, wts


if __name__ == "__main__":
    rng = np.random.default_rng(0)
    x = rng.standard_normal((B, S, H), dtype=np.float32)
    w = rng.standard_normal(H, dtype=np.float32) * (1.0 / np.sqrt(H))
    b = np.float32(0.01)
    ctx, wts = kernel(x, w, b)
    print(ctx.shape, wts.shape)
